# revision 20
# baseline (speedup 1.0000x reference)
"""Trainium2 Bass kernel for an RNN-T style joint network MLP.

  out[b,t,u,o] = tanh(enc[b,t,:] @ W1[:512] + dec[b,u,:] @ W1[512:] + b1) @ W2 + b2

Shapes: enc (8, 256, 512), dec (8, 64, 512), W1 (1024, 1024), b1 (1024,),
W2 (1024, 128), b2 (128,), out (8, 256, 64, 128), all float32.

Sharding: data-parallel over batch — one batch element per NeuronCore, no
collectives.  The kernel is elementwise-bound: 16.8M hidden elements per core
need a broadcast-add (DVE tensor_scalar, 2x bf16) and a tanh (ACT, 1
elem/cycle/lane).  Steady state balances ACT ~= DVE ~= 119us busy:
  - ACT: big per-block tanh ops + a few PSUM evacuations + head e_proj evacs
  - DVE: all 512 broadcast-adds + most PSUM evacuations (+b2)
  - PE:  enc/dec projections, then the main GEMM (N=512 per u-pair)
Head tricks: dma_start issues spread over idle engine queues (each issue
costs ~0.65us serially on its queue), host-side pre-swizzled input layouts
(2KB-contiguous per partition row -> fewest DMA descriptors), and dummy PE
warm-up matmuls so the HAM clock gate reaches 2.4GHz before the real GEMM.
Tail trick: the last u-block is split in two UB=2 halves so the final
tanh->GEMM->evac->DMA chain is short.

GPSIMD broadcast-adds and a custom deg-7 polynomial tanh on DVE were tried
and rejected: GPSIMD shares its SBUF port with the DVE and the two engines
serialize (measured), and the poly-tanh trade (1.25ns/elem DVE for
0.88ns/elem ACT) is worse than moving evacuations to ACT.
"""

import os
import numpy as np
import ml_dtypes

B, T, U, D, H, O = 8, 256, 64, 512, 1024, 128
NCORES = 8
HC = H // 128     # 8 h-chunks

# u-block sizes (pipeline granularity); last block split for a short tail
UBS = [4] * 15 + [2, 2]
# blocks whose PSUM evacuation (+b2) runs on ACT instead of DVE (balance knob;
# early blocks, where DVE is still ramping the add pipeline)
ACT_EVAC_BLOCKS = frozenset({1, 6, 11})
# h-chunks whose e_proj PSUM evac runs on ACT (rest on DVE)
ACT_EEVAC_HCS = frozenset(range(5))
N_WARMUP_MM = 4   # dummy matmuls to lift the PE HAM clock gate before the GEMM

_CACHE = {}
LAST_RESULT = None  # BassKernelResults from the most recent run (for profiling)


def _build_program():
    from concourse import bacc, tile
    import concourse.mybir as mybir

    dt = mybir.dt
    f32, bf16 = dt.float32, dt.bfloat16
    Act = mybir.ActivationFunctionType

    nc = bacc.Bacc("TRN2", target_bir_lowering=False, debug=False)

    # host-side pre-swizzled layouts: every dram row maps to one partition row
    # with a 2KB contiguous extent (fewest DMA descriptors)
    encTr = nc.dram_tensor("encTr", [128, 4 * T], bf16, kind="ExternalInput").ap()
    decTr = nc.dram_tensor("decTr", [128, 4 * U], bf16, kind="ExternalInput").ap()
    # W1 host-swizzled hc-major: W1e_r[p, hc*512 + dc*128 + j] = W1[dc*128+p, hc*128+j]
    # so each per-hc dma_start (128KB) unblocks that h-chunk's first GEMM.
    W1e = nc.dram_tensor("W1e", [128, HC * 512], bf16, kind="ExternalInput").ap()
    W1d = nc.dram_tensor("W1d", [128, HC * 512], bf16, kind="ExternalInput").ap()
    W2r = nc.dram_tensor("W2r", [128, HC * O], bf16, kind="ExternalInput").ap()
    b1r = nc.dram_tensor("b1r", [128, HC], f32, kind="ExternalInput").ap()
    b2c = nc.dram_tensor("b2c", [O, 1], f32, kind="ExternalInput").ap()
    outT = nc.dram_tensor("outT", [O, U, T], f32, kind="ExternalOutput").ap()

    with tile.TileContext(nc) as tc:
        with tc.tile_pool(name="persist", bufs=1) as persist, \
             tc.tile_pool(name="sums", bufs=3) as sums_pool, \
             tc.tile_pool(name="tanhp", bufs=3) as tanh_pool, \
             tc.tile_pool(name="outsb", bufs=3) as out_pool, \
             tc.tile_pool(name="hpsum", bufs=2, space="PSUM") as hpsum_pool, \
             tc.tile_pool(name="psum", bufs=3, space="PSUM") as psum_pool:

            w1e_sb = persist.tile([128, HC * 512], bf16, tag="w1e")
            w1d_sb = persist.tile([128, HC * 512], bf16, tag="w1d")
            encT_sb = persist.tile([128, 4 * T], bf16, tag="encT")
            decT_sb = persist.tile([128, 4 * U], bf16, tag="decT")
            w2_sb = persist.tile([128, HC * O], bf16, tag="w2")
            b1_sb = persist.tile([128, HC], f32, tag="b1")
            b2_sb = persist.tile([128, 1], f32, tag="b2")
            e_sb = persist.tile([128, HC * T], bf16, tag="eproj")
            bias_sb = persist.tile([128, HC * U], f32, tag="bias")
            scr_sb = persist.tile([128, 512], bf16, tag="scratch")

            # ---- PE warm-up: dummy matmuls on scratch data keep the PE busy
            # from t~7us so the HAM clock gate is at 2.4GHz when the real
            # GEMM starts (saves ~3us of half-clock matmuls at the head).
            nc.vector.memset(scr_sb[:], 0.0)
            pw = hpsum_pool.tile([128, 512], f32, tag="ps", name="warm")
            for i in range(N_WARMUP_MM):
                nc.tensor.matmul(pw[:], lhsT=scr_sb[:, 0:128], rhs=scr_sb[:],
                                 start=True, stop=True)

            # ---- loads: DMA descriptors spray across all 16 rings, so the
            # head is bandwidth-bound (~2.4MB at ~290GB/s = 8.5us).  W1 is
            # loaded hc-sliced (host-swizzled) so hc0's 256KB lands in ~1us
            # and the first GEMM pipelines with the rest of the load.  Issues
            # come from three engine queues (SP/ACT HWDGE + gpsimd SWDGE) so
            # their ~0.65us per-call issue cost is paid in parallel.
            # hc0 slices first (256KB unblocks the first h-chunk's GEMM in
            # ~1.5us), then growing slices pipeline behind.  The Scalar queue
            # issues nothing so e_proj evacs aren't stuck behind DIRECT2Ds.
            nc.sync.dma_start(encT_sb[:], encTr[:, :])
            nc.sync.dma_start(w1e_sb[:, 0:512], W1e[:, 0:512])
            nc.sync.dma_start(w1d_sb[:, 0:512], W1d[:, 0:512])
            nc.sync.dma_start(w1e_sb[:, 512:4 * 512], W1e[:, 512:4 * 512])
            nc.sync.dma_start(w1d_sb[:, 512:4 * 512], W1d[:, 512:4 * 512])
            nc.sync.dma_start(w1e_sb[:, 4 * 512:8 * 512], W1e[:, 4 * 512:8 * 512])
            nc.gpsimd.dma_start(b1_sb[:], b1r[:, :])
            nc.gpsimd.dma_start(decT_sb[:], decTr[:, :])
            nc.gpsimd.dma_start(w1d_sb[:, 4 * 512:8 * 512],
                                W1d[:, 4 * 512:8 * 512])
            nc.gpsimd.dma_start(w2_sb[:], W2r[:, :])
            nc.gpsimd.dma_start(b2_sb[:], b2c[:, :])

            # ---- first GEMMs, interleaved per h-chunk so downstream adds can
            # start on hc0 while hc1.. are still multiplying.
            # enc: e_projT[h,t] = sum_d W_enc[d,h]*encT[d,t]
            # dec: bias[h,u] = sum_d W_dec[d,h]*decT[d,u] + b1 (evac on DVE)
            for hc in range(HC):
                pe = hpsum_pool.tile([128, T], f32, tag="ps", name=f"pe{hc}")
                for dc in range(4):
                    nc.tensor.matmul(
                        pe[:],
                        lhsT=w1e_sb[:, hc * 512 + dc * 128: hc * 512 + dc * 128 + 128],
                        rhs=encT_sb[:, dc * T:(dc + 1) * T],
                        start=(dc == 0), stop=(dc == 3),
                    )
                if hc in ACT_EEVAC_HCS:
                    nc.scalar.activation(e_sb[:, hc * T:(hc + 1) * T], pe[:],
                                         Act.Identity)
                else:
                    nc.vector.tensor_copy(e_sb[:, hc * T:(hc + 1) * T], pe[:])

                pd = hpsum_pool.tile([128, U], f32, tag="ps", name=f"pd{hc}")
                for dc in range(4):
                    nc.tensor.matmul(
                        pd[:],
                        lhsT=w1d_sb[:, hc * 512 + dc * 128: hc * 512 + dc * 128 + 128],
                        rhs=decT_sb[:, dc * U:(dc + 1) * U],
                        start=(dc == 0), stop=(dc == 3),
                    )
                nc.vector.tensor_scalar_add(bias_sb[:, hc * U:(hc + 1) * U],
                                            pd[:], b1_sb[:, hc:hc + 1])

            # ---- main pipeline over u-blocks ----
            # sum/tanh layout per block: [hc][u][t] (hc-major); the main GEMM
            # runs N=512 per u-pair into one 1-2 bank PSUM tile.
            u0 = 0
            for blk, ub in enumerate(UBS):
                bw = ub * 2048      # block free width
                hcw = ub * T        # per-(block, hc) width

                sum_sb = sums_pool.tile([128, bw], bf16, tag="sum")
                for hc in range(HC):
                    for ul in range(ub):
                        nc.vector.tensor_scalar_add(
                            sum_sb[:, hc * hcw + ul * T: hc * hcw + ul * T + T],
                            e_sb[:, hc * T:(hc + 1) * T],
                            bias_sb[:, hc * U + u0 + ul: hc * U + u0 + ul + 1],
                        )

                tanh_sb = tanh_pool.tile([128, bw], bf16, tag="tanh")
                # split tanh at the pipeline head (a slice needs only 1-2
                # h-chunks of adds -> chases the W1 load + first GEMM) and
                # tail (lets the PE chase the drain)
                nsplit = (8 if blk == 0 else 4 if blk == 1 else
                          2 if blk in (2, len(UBS) - 2, len(UBS) - 1) else 1)
                for q in range(nsplit):
                    nc.scalar.activation(
                        tanh_sb[:, q * bw // nsplit:(q + 1) * bw // nsplit],
                        sum_sb[:, q * bw // nsplit:(q + 1) * bw // nsplit],
                        Act.Tanh)

                npair = ub // 2
                po = psum_pool.tile([128, npair * 2 * T], f32, tag="ps",
                                    name=f"po{blk}")
                for hc in range(HC):  # hc outer: W2 chunk stays stationary
                    for p in range(npair):
                        nc.tensor.matmul(
                            po[:, p * 2 * T:(p + 1) * 2 * T],
                            lhsT=w2_sb[:, hc * O:(hc + 1) * O],
                            rhs=tanh_sb[:, hc * hcw + p * 2 * T: hc * hcw + (p + 1) * 2 * T],
                            start=(hc == 0), stop=(hc == HC - 1),
                        )

                out_sb = out_pool.tile([128, ub * T], f32, tag="osb")
                if blk == len(UBS) - 1:
                    # final block: evac+store per u so the drain is short
                    for j in range(ub):
                        nc.vector.tensor_scalar_add(
                            out_sb[:, j * T:(j + 1) * T],
                            po[:, j * T:(j + 1) * T], b2_sb[:, 0:1])
                        nc.sync.dma_start(outT[:, u0 + j:u0 + j + 1, :],
                                          out_sb[:, j * T:(j + 1) * T])
                else:
                    if blk in ACT_EVAC_BLOCKS:
                        nc.scalar.activation(out_sb[:], po[:], Act.Identity,
                                             bias=b2_sb[:, 0:1])
                    else:
                        nc.vector.tensor_scalar_add(out_sb[:], po[:],
                                                    b2_sb[:, 0:1])
                    nc.sync.dma_start(outT[:, u0:u0 + ub, :], out_sb[:])
                u0 += ub

    nc.compile()
    return nc


def _host_inputs(enc_i, dec_i, b1r, b2c):
    """Per-core input map with pre-swizzled layouts (2KB/partition rows)."""
    bf = ml_dtypes.bfloat16
    # encTr[p, c*T+t] = enc[t, c*128+p]
    encT = np.ascontiguousarray(enc_i.T.astype(bf))          # [512, 256]
    encTr = np.ascontiguousarray(
        encT.reshape(4, 128, T).transpose(1, 0, 2).reshape(128, 4 * T))
    decT = np.ascontiguousarray(dec_i.T.astype(bf))          # [512, 64]
    decTr = np.ascontiguousarray(
        decT.reshape(4, 128, U).transpose(1, 0, 2).reshape(128, 4 * U))
    return {"encTr": encTr, "decTr": decTr, "b1r": b1r, "b2c": b2c}


def _host_weights(W1, W2, bf):
    """W1e/W1d hc-major swizzles + W2r."""
    # W1e[p, hc*512 + dc*128 + j] = W1[dc*128 + p, hc*128 + j]
    We = W1[:D].astype(bf).reshape(4, 128, HC, 128)
    W1e = np.ascontiguousarray(We.transpose(1, 2, 0, 3).reshape(128, HC * 512))
    Wd = W1[D:].astype(bf).reshape(4, 128, HC, 128)
    W1d = np.ascontiguousarray(Wd.transpose(1, 2, 0, 3).reshape(128, HC * 512))
    W2r = np.ascontiguousarray(
        W2.astype(bf).reshape(HC, 128, O).transpose(1, 0, 2).reshape(128, HC * O))
    return W1e, W1d, W2r


def kernel(encoder_state, decoder_state, W1, b1, W2, b2):
    from concourse.bass_utils import run_bass_kernel_spmd
    global LAST_RESULT

    if "nc" not in _CACHE:
        _CACHE["nc"] = _build_program()
    nc = _CACHE["nc"]

    encoder_state = np.asarray(encoder_state, dtype=np.float32)
    decoder_state = np.asarray(decoder_state, dtype=np.float32)
    W1 = np.asarray(W1, dtype=np.float32)
    b1 = np.asarray(b1, dtype=np.float32)
    W2 = np.asarray(W2, dtype=np.float32)
    b2 = np.asarray(b2, dtype=np.float32)

    bf = ml_dtypes.bfloat16
    W1e, W1d, W2r = _host_weights(W1, W2, bf)
    b1r = np.ascontiguousarray(b1.reshape(HC, 128).T)  # [128, 8]
    b2c = np.ascontiguousarray(b2.reshape(O, 1))

    in_maps = []
    for i in range(NCORES):
        m = _host_inputs(encoder_state[i], decoder_state[i], b1r, b2c)
        m.update({"W1e": W1e, "W1d": W1d, "W2r": W2r})
        in_maps.append(m)

    trace = bool(int(os.environ.get("KERNEL_TRACE", "0")))
    res = run_bass_kernel_spmd(nc, in_maps, list(range(NCORES)), trace=trace)
    LAST_RESULT = res

    # gather: outT[core] is [O, U, T] -> out[b, t, u, o]
    out = np.empty((B, T, U, O), dtype=np.float32)
    for i in range(NCORES):
        out[i] = res.results[i]["outT"].transpose(2, 1, 0)
    return out
